# revision 12
# baseline (speedup 1.0000x reference)
"""Balanced CE loss + accuracy on 8 Trainium2 NeuronCores (Bass/Tile).

Reference computation (N = 16777216 elements):
    loss = -sum(where(t==1, 1.6*log(p), 0.4*log(1-p))) / N
    acc  = mean(round(p) == t)

Strategy (data-parallel over N, no collectives):
  Shard N across 8 cores; per core stream [128, C] sub-chunks.  The
  per-element work is balanced across four engines so the kernel is
  paced by the irreducible HBM traffic (8 B/elem):

    ACT   u = ln(p) -> bf16, v = ln(1-p) -> bf16 (scale=-1, bias=1;
          accum gives sum(v)).  Biases are tracked tiles, not float
          consts, so no const-AP preamble barrier delays the DMAs.
    Pool  tb = bf16(t) cast, accum gives sum(t).  (DVE 2-input ops
          never get a perf mode, so folding t into p on DVE costs 2x;
          multiplying AFTER the log keeps every product bf16/2x and
          the cast runs on the otherwise idle GpSimd engine.)
    DVE   x1 = u*tb, x0 = v*tb (bf16 tensor_tensor, 2x mode) and two
          bf16 is_ge mask passes (4x mode) whose accum_out yields
            E1 = #(x1 >= ln .5) = N0 + #(t==1 & p>=.5)
            E2 = #(u  >= ln .5) = #(p >= .5)
    PE    ones^T @ x1 / x0 accumulate column sums in two PSUM banks
          (32 matmuls each); folded to scalars at the end.

  Host combine (f64):  A1 = sum(x1) = sum_{t==1} ln p,
    B0 = sum(v) - sum(x0) = sum_{t==0} ln(1-p),
    loss = -(1.6*A1 + 0.4*B0)/N,
    correct = 2*E1 + sum(t) - E2 - N  (since correct = t*m + (1-t)(1-m)
    with m = [p>=.5]),  acc = correct/N.

  bf16 rounding only ever touches ln values AFTER they are computed in
  f32 (no catastrophic cancellation); numpy emulation puts the combined
  error at ~1e-5 relative, vs the 2e-2 gate.
"""

import sys

if "/opt/trn_rl_repo" not in sys.path:
    sys.path.insert(0, "/opt/trn_rl_repo")

import numpy as np

import concourse.bass as bass
import concourse.bacc as bacc
import concourse.tile as tile
from concourse import mybir
from concourse.bass_utils import run_bass_kernel_spmd

N_CORES = 8
N = 16777216
P = 128
SHARD = N // N_CORES          # 2097152 elements per core
COLS = SHARD // P             # 16384 columns per core
# first sub-chunk split in half so ACT/DVE start ~2us earlier
SUBS = [1024, 1024] + [2048] * 7
NS = len(SUBS)
MMCOL = 512                   # matmul free-dim tile (one PSUM bank)
TH = -0.6931471805599453      # ln(0.5)

AF = mybir.ActivationFunctionType
OP = mybir.AluOpType

# acc tile columns: [0,NS) sum(v), [NS,2NS) E1, [2NS,3NS) E2,
# [3NS] sum(x1) fold, [3NS+1] sum(x0) fold, [3NS+2] sum(t) fold
ACC_COLS = 3 * NS + 3

_NC_CACHE = None


def build_bass():
    """Build the single-core Bass program (SPMD across 8 cores)."""
    global _NC_CACHE
    if _NC_CACHE is not None:
        return _NC_CACHE

    nc = bacc.Bacc("TRN2", target_bir_lowering=False, debug=False)

    p_in = nc.dram_tensor("p_in", [SHARD], mybir.dt.float32, kind="ExternalInput").ap()
    t_in = nc.dram_tensor("t_in", [SHARD], mybir.dt.int32, kind="ExternalInput").ap()
    acc_out = nc.dram_tensor("acc_out", [P, ACC_COLS], mybir.dt.float32, kind="ExternalOutput").ap()

    n_mm = COLS // MMCOL          # 32 matmuls per reduced quantity

    with tile.TileContext(nc) as tc:
        with (
            tc.tile_pool(name="io", bufs=NS - 1) as io_pool,
            tc.tile_pool(name="lg", bufs=3) as lg_pool,
            tc.tile_pool(name="pr", bufs=3) as pr_pool,
            tc.tile_pool(name="jk", bufs=1) as jk_pool,
            tc.tile_pool(name="ps", bufs=1, space=bass.MemorySpace.PSUM) as psum_pool,
            tc.tile_pool(name="mi", bufs=1) as misc_pool,
        ):
            ones16 = misc_pool.tile([P, P], mybir.dt.bfloat16, tag="ones16")
            nc.gpsimd.memset(ones16[:], 1.0)
            cz = misc_pool.tile([P, 1], mybir.dt.float32, tag="cz")
            co = misc_pool.tile([P, 1], mybir.dt.float32, tag="co")
            nc.gpsimd.memset(cz[:], 0.0)
            nc.gpsimd.memset(co[:], 1.0)
            warm = misc_pool.tile([P, 1], mybir.dt.float32, tag="warm")
            acc = misc_pool.tile([P, ACC_COLS], mybir.dt.float32, tag="acc")
            junk512 = misc_pool.tile([P, MMCOL], mybir.dt.float32, tag="junk512")
            psA = psum_pool.tile([P, MMCOL], mybir.dt.float32, tag="psA")
            psB = psum_pool.tile([P, MMCOL], mybir.dt.float32, tag="psB")
            psC = psum_pool.tile([P, MMCOL], mybir.dt.float32, tag="psC")

            # load the Ln table while the first DMA is in flight
            nc.scalar.activation(warm[:, 0:1], co[:, 0:1], AF.Ln, bias=cz[:, 0:1])

            mmA = mmB = mmC = 0
            off = 0
            for s, C in enumerate(SUBS):
                p_t = io_pool.tile([P, C], mybir.dt.float32, tag="p")
                t_t = io_pool.tile([P, C], mybir.dt.int32, tag="t")
                nc.sync.dma_start(p_t[:], p_in[off : off + C * P].rearrange("(p f) -> p f", p=P))
                nc.sync.dma_start(t_t[:], t_in[off : off + C * P].rearrange("(p f) -> p f", p=P))
                off += C * P

                ub = lg_pool.tile([P, C], mybir.dt.bfloat16, tag="ub")
                vb = lg_pool.tile([P, C], mybir.dt.bfloat16, tag="vb")
                tb = lg_pool.tile([P, C], mybir.dt.bfloat16, tag="tb")
                nc.vector.tensor_copy(tb[:], t_t[:])
                nc.scalar.activation(ub[:], p_t[:], AF.Ln, bias=cz[:, 0:1])
                nc.scalar.activation(vb[:], p_t[:], AF.Ln, bias=co[:, 0:1], scale=-1.0,
                                     accum_out=acc[:, s : s + 1])

                x1 = pr_pool.tile([P, C], mybir.dt.bfloat16, tag="x1")
                x0 = pr_pool.tile([P, C], mybir.dt.bfloat16, tag="x0")
                nc.vector.tensor_tensor(x1[:], ub[:], tb[:], OP.mult)
                nc.vector.tensor_tensor(x0[:], vb[:], tb[:], OP.mult)

                j1 = jk_pool.tile([P, C], mybir.dt.bfloat16, tag="j1")
                j2 = jk_pool.tile([P, C], mybir.dt.bfloat16, tag="j2")
                nc.vector.tensor_scalar(j1[:], x1[:], TH, None, OP.is_ge, OP.add,
                                        accum_out=acc[:, NS + s : NS + s + 1])
                nc.vector.tensor_scalar(j2[:], ub[:], TH, None, OP.is_ge, OP.add,
                                        accum_out=acc[:, 2 * NS + s : 2 * NS + s + 1])

                for j in range(C // MMCOL):
                    nc.tensor.matmul(psA[:], ones16[:], x1[:, j * MMCOL : (j + 1) * MMCOL],
                                     start=(mmA == 0), stop=(mmA == n_mm - 1))
                    mmA += 1
                    nc.tensor.matmul(psB[:], ones16[:], x0[:, j * MMCOL : (j + 1) * MMCOL],
                                     start=(mmB == 0), stop=(mmB == n_mm - 1))
                    mmB += 1
                    nc.tensor.matmul(psC[:], ones16[:], tb[:, j * MMCOL : (j + 1) * MMCOL],
                                     start=(mmC == 0), stop=(mmC == n_mm - 1))
                    mmC += 1

            nc.vector.tensor_scalar(junk512[:], psA[:], 1.0 / P, None, OP.mult,
                                    OP.add, accum_out=acc[:, 3 * NS : 3 * NS + 1])
            nc.vector.tensor_scalar(junk512[:], psB[:], 1.0 / P, None, OP.mult,
                                    OP.add, accum_out=acc[:, 3 * NS + 1 : 3 * NS + 2])
            nc.vector.tensor_scalar(junk512[:], psC[:], 1.0 / P, None, OP.mult,
                                    OP.add, accum_out=acc[:, 3 * NS + 2 : 3 * NS + 3])

            nc.sync.dma_start(acc_out[:], acc[:])

    nc.finalize()
    _NC_CACHE = nc
    return nc


def make_in_maps(input, target):
    inp = np.ascontiguousarray(np.asarray(input, dtype=np.float32)).reshape(
        N_CORES, SHARD
    )
    tgt = np.ascontiguousarray(np.asarray(target, dtype=np.int32)).reshape(
        N_CORES, SHARD
    )
    return [{"p_in": inp[c], "t_in": tgt[c]} for c in range(N_CORES)]


def combine(results):
    """Host-side unshard: reduce the 8 cores' partial sums -> (loss, acc)."""
    wsum = 0.0
    cnt = 0.0
    for r in results:
        a = np.asarray(r["acc_out"], dtype=np.float64)
        sumV = a[:, 0:NS].sum()
        E1 = a[:, NS : 2 * NS].sum()
        E2 = a[:, 2 * NS : 3 * NS].sum()
        A1 = a[:, 3 * NS].sum()
        X0 = a[:, 3 * NS + 1].sum()
        sumT = a[:, 3 * NS + 2].sum()
        B0 = sumV - X0
        wsum += 1.6 * A1 + 0.4 * B0
        cnt += 2.0 * E1 + sumT - E2 - SHARD
    loss = -wsum / N
    acc = cnt / N
    return np.float32(loss), np.float32(acc)


def run_on_hw(input, target, **spmd_kwargs):
    nc = build_bass()
    in_maps = make_in_maps(input, target)
    return run_bass_kernel_spmd(nc, in_maps, list(range(N_CORES)), **spmd_kwargs)


def kernel(input, target):
    br = run_on_hw(input, target)
    return combine(br.results)
